# revision 4
# baseline (speedup 1.0000x reference)
"""Trainium2 Bass kernel: causal multi-head attention block (B=2, S=2048, D=2048, H=16).

Sharding: 8 cores = 2 (batch) x 4 (head-groups of 4 heads). Each core computes
its batch's attention output restricted to its 4 heads plus the corresponding
partial out-projection; the host sums the 4 head-group partials per batch and
adds the (o_b + o_w @ v_b) bias vector (valid because softmax rows sum to 1).

All on-device matmuls run in float32r (TF32-like, full PE rate at N=512) with
fp32 PSUM accumulation. Layout is fully transposed-friendly: Q^T/K^T are
produced as [d, s] directly, scores are computed transposed as [keys, q] so
softmax's exp feeds P^T straight into the attn@V and row-sum matmuls with no
on-device transposes at all. Normalization (divide by softmax denominator) is
applied to the [d, q] attention output via a K=1 broadcast matmul + multiply.
"""

import sys

sys.path.insert(0, "/opt/trn_rl_repo")

import numpy as np
import concourse.bacc as bacc
import concourse.tile as tile
from concourse import mybir
from concourse.bass_utils import run_bass_kernel_spmd

F32 = mybir.dt.float32
F32R = mybir.dt.float32r

B, S, D, H, HD = 2, 2048, 2048, 16, 128
SCALE = 1.0 / (HD**0.5)
HL = 4  # heads per core
DL = HL * HD  # 512: local head dims per core
NK = D // HD  # 16 contraction k-tiles
NJ = S // DL  # 4 blocks of 512 along sequence
NEG = -1.0e30

_CACHE = {}


def _build():
    nc = bacc.Bacc("TRN2", target_bir_lowering=False, debug=False)
    ExpF = mybir.ActivationFunctionType.Exp

    xT = nc.declare_dram_parameter("xT", [D, S], F32, isOutput=False)
    wq = nc.declare_dram_parameter("wq", [D, DL], F32, isOutput=False)
    wk = nc.declare_dram_parameter("wk", [D, DL], F32, isOutput=False)
    wv = nc.declare_dram_parameter("wv", [D, DL], F32, isOutput=False)
    wo = nc.declare_dram_parameter("wo", [DL, D], F32, isOutput=False)
    bq = nc.declare_dram_parameter("bq", [HD, HL], F32, isOutput=False)
    bk = nc.declare_dram_parameter("bk", [HD, HL], F32, isOutput=False)
    maskT = nc.declare_dram_parameter("maskT", [DL, DL], F32, isOutput=False)
    onec = nc.declare_dram_parameter("onec", [HD, 1], F32, isOutput=False)
    oner = nc.declare_dram_parameter("oner", [1, HD], F32, isOutput=False)
    out = nc.declare_dram_parameter("out", [S, D], F32, isOutput=True)

    with tile.TileContext(nc) as tc:
        with (
            tc.tile_pool(name="const", bufs=1) as constp,
            tc.tile_pool(name="qk", bufs=1) as qkp,
            tc.tile_pool(name="vres", bufs=1) as vp,
            tc.tile_pool(name="wop", bufs=1) as wop,
            tc.tile_pool(name="xs", bufs=6) as xs,
            tc.tile_pool(name="ws", bufs=2) as ws,
            tc.tile_pool(name="pt", bufs=6) as ptp,
            tc.tile_pool(name="attn", bufs=8) as attnp,
            tc.tile_pool(name="norm", bufs=2) as normp,
            tc.tile_pool(name="ob", bufs=3) as obp,
            tc.tile_pool(name="ps", bufs=8, space="PSUM") as ps,
        ):
            # --- constants ---
            bq_sb = constp.tile([HD, HL], F32, tag="bq")
            nc.sync.dma_start(bq_sb[:], bq[:, :])
            bk_sb = constp.tile([HD, HL], F32, tag="bk")
            nc.sync.dma_start(bk_sb[:], bk[:, :])
            onec_sb = constp.tile([HD, 1], F32R, tag="onec")
            nc.sync.dma_start(onec_sb[:], onec[:, :].bitcast(F32R))
            oner_sb = constp.tile([1, HD], F32R, tag="oner")
            nc.sync.dma_start(oner_sb[:], oner[:, :].bitcast(F32R))
            mask_sb = []
            for r in range(4):
                m = constp.tile([HD, DL], F32, tag=f"mask{r}")
                nc.sync.dma_start(m[:], maskT[r * HD : (r + 1) * HD, :])
                mask_sb.append(m)
            wo_sb = []
            for dh in range(HL):
                w = wop.tile([HD, D], F32R, tag=f"wo{dh}")
                nc.sync.dma_start(w[:], wo[dh * HD : (dh + 1) * HD, :].bitcast(F32R))
                wo_sb.append(w)

            # --- residents ---
            QT = [qkp.tile([HD, S], F32R, tag=f"qt{h}", name=f"qt{h}") for h in range(HL)]
            KT = [qkp.tile([HD, S], F32R, tag=f"kt{h}", name=f"kt{h}") for h in range(HL)]
            V = [vp.tile([HD, DL], F32R, tag=f"v{t}", name=f"v{t}") for t in range(S // HD)]

            # --- phase A: projections ---
            for J in range(NJ):
                sl_s = slice(DL * J, DL * (J + 1))
                # A1: Q^T and K^T for this 512-wide s-block, all 4 heads
                qps = [ps.tile([HD, DL], F32, tag="ps", name=f"qps{J}_{i}") for i in range(HL)]
                kps = [ps.tile([HD, DL], F32, tag="ps", name=f"kps{J}_{i}") for i in range(HL)]
                for k in range(NK):
                    sl_k = slice(HD * k, HD * (k + 1))
                    xt = xs.tile([HD, DL], F32R, tag="xt")
                    nc.sync.dma_start(xt[:], xT[sl_k, sl_s].bitcast(F32R))
                    wqt = ws.tile([HD, DL], F32R, tag="wq")
                    nc.sync.dma_start(wqt[:], wq[sl_k, :].bitcast(F32R))
                    wkt = ws.tile([HD, DL], F32R, tag="wk")
                    nc.sync.dma_start(wkt[:], wk[sl_k, :].bitcast(F32R))
                    for h in range(HL):
                        sl_h = slice(HD * h, HD * (h + 1))
                        nc.tensor.matmul(
                            qps[h][:], wqt[:, sl_h], xt[:],
                            start=(k == 0), stop=(k == NK - 1),
                        )
                        nc.tensor.matmul(
                            kps[h][:], wkt[:, sl_h], xt[:],
                            start=(k == 0), stop=(k == NK - 1),
                        )
                for h in range(HL):
                    nc.scalar.add(QT[h][:, sl_s], qps[h][:], bq_sb[:, h : h + 1])
                    nc.scalar.add(KT[h][:, sl_s], kps[h][:], bk_sb[:, h : h + 1])
                # A2: V natural layout for the 4 s-tiles of this block
                vps = [ps.tile([HD, DL], F32, tag="ps", name=f"vps{J}_{i}") for i in range(4)]
                for k in range(NK):
                    sl_k = slice(HD * k, HD * (k + 1))
                    xt2 = xs.tile([HD, DL], F32R, tag="xt")
                    nc.sync.dma_start(xt2[:], xT[sl_k, sl_s].bitcast(F32R))
                    wvt = ws.tile([HD, DL], F32R, tag="wv")
                    nc.sync.dma_start(wvt[:], wv[sl_k, :].bitcast(F32R))
                    for t in range(4):
                        nc.tensor.matmul(
                            vps[t][:], xt2[:, HD * t : HD * (t + 1)], wvt[:],
                            start=(k == 0), stop=(k == NK - 1),
                        )
                for t in range(4):
                    nc.scalar.copy(V[4 * J + t][:], vps[t][:])

            # --- phases B (attention) + C (out-proj), per 512-wide q-block ---
            for J in range(NJ):
                sl_q = slice(DL * J, DL * (J + 1))
                attn_t = []
                for h in range(HL):
                    sl_h = slice(HD * h, HD * (h + 1))
                    nkt = 4 * (J + 1)  # causal: key tiles 0..nkt-1
                    aps = ps.tile([HD, DL], F32, tag="ps")
                    sps = ps.tile([HD, DL], F32, tag="ps")
                    for i in range(nkt):
                        scp = ps.tile([HD, DL], F32, tag="ps")
                        nc.tensor.matmul(
                            scp[:], KT[h][:, HD * i : HD * (i + 1)], QT[h][:, sl_q],
                            start=True, stop=True,
                        )
                        if i >= 4 * J:
                            nc.vector.tensor_add(scp[:], scp[:], mask_sb[i - 4 * J][:])
                        ptt = ptp.tile([HD, DL], F32R, tag="pt")
                        nc.scalar.activation(ptt[:], scp[:], ExpF)
                        nc.tensor.matmul(
                            aps[:], V[i][:, sl_h], ptt[:],
                            start=(i == 0), stop=(i == nkt - 1),
                        )
                        nc.tensor.matmul(
                            sps[0:1, :], onec_sb[:], ptt[:],
                            start=(i == 0), stop=(i == nkt - 1),
                        )
                    rs = normp.tile([1, DL], F32R, tag="rs")
                    with nc.allow_low_precision(reason="f32r is 32-bit storage; rounding only"):
                        nc.vector.reciprocal(rs[:], sps[0:1, :])
                    bcp = ps.tile([HD, DL], F32, tag="ps")
                    nc.tensor.matmul(bcp[:], oner_sb[:], rs[:], start=True, stop=True)
                    bcs = normp.tile([HD, DL], F32, tag="bc")
                    nc.scalar.copy(bcs[:], bcp[:])
                    at = attnp.tile([HD, DL], F32R, tag="at")
                    nc.vector.tensor_mul(at[:], aps[:], bcs[:])
                    attn_t.append(at)
                # C: out-projection for the 4 s-tiles of this q-block
                for c in range(4):
                    sl_c = slice(HD * c, HD * (c + 1))
                    st = 4 * J + c
                    for nb in range(4):
                        sl_n = slice(DL * nb, DL * (nb + 1))
                        op = ps.tile([HD, DL], F32, tag="ps")
                        for dh in range(HL):
                            nc.tensor.matmul(
                                op[:], attn_t[dh][:, sl_c], wo_sb[dh][:, sl_n],
                                start=(dh == 0), stop=(dh == HL - 1),
                            )
                        ob = obp.tile([HD, DL], F32, tag="ob")
                        nc.scalar.copy(ob[:], op[:])
                        nc.sync.dma_start(out[HD * st : HD * (st + 1), sl_n], ob[:])

    nc.compile()
    return nc


def _prep_in_maps(x, q_w, q_b, k_w, k_b, v_w, v_b, o_w, o_b):
    maskT = np.where(
        np.arange(DL)[:, None] > np.arange(DL)[None, :], np.float32(NEG), np.float32(0)
    ).astype(np.float32)
    onec = np.ones((HD, 1), np.float32)
    oner = np.ones((1, HD), np.float32)
    in_maps = []
    for c in range(8):
        b, hg = divmod(c, 4)
        ds = slice(DL * hg, DL * (hg + 1))
        in_maps.append(
            {
                "xT": np.ascontiguousarray(x[b].T),
                "wq": np.ascontiguousarray(q_w[ds].T * SCALE),
                "wk": np.ascontiguousarray(k_w[ds].T),
                "wv": np.ascontiguousarray(v_w[ds].T),
                "wo": np.ascontiguousarray(o_w[:, ds].T),
                "bq": np.ascontiguousarray((q_b[ds] * SCALE).reshape(HL, HD).T),
                "bk": np.ascontiguousarray(k_b[ds].reshape(HL, HD).T),
                "maskT": maskT,
                "onec": onec,
                "oner": oner,
            }
        )
    return in_maps


def kernel(x, q_w, q_b, k_w, k_b, v_w, v_b, o_w, o_b, _trace=False, _trace_kwargs=None):
    x = np.asarray(x, np.float32)
    args = [np.asarray(a, np.float32) for a in (q_w, q_b, k_w, k_b, v_w, v_b, o_w, o_b)]
    q_w, q_b, k_w, k_b, v_w, v_b, o_w, o_b = args

    if "nc" not in _CACHE:
        _CACHE["nc"] = _build()
    nc = _CACHE["nc"]

    in_maps = _prep_in_maps(x, q_w, q_b, k_w, k_b, v_w, v_b, o_w, o_b)
    res = run_bass_kernel_spmd(
        nc, in_maps, list(range(8)), trace=_trace, **(_trace_kwargs or {})
    )
    _CACHE["last_result"] = res

    bias_vec = (o_w @ v_b + o_b).astype(np.float32)
    out = np.empty((B, S, D), np.float32)
    for b in range(B):
        acc = res.results[4 * b]["out"].astype(np.float32).copy()
        for hg in range(1, 4):
            acc += res.results[4 * b + hg]["out"]
        out[b] = acc + bias_vec
    return out
